# revision 43
# baseline (speedup 1.0000x reference)
"""FP8Linear (blockwise fp8 quant-dequant + matmul) Trainium2 Bass kernel.

Reference semantics (per 128-contiguous-element block, flattened):
    s = max|block| / 448 ; q = fp8_e4m3fn(block / s) ; deq = q * s
    out = x_deq @ w_deq.T

Strategy (v5, ~160us/core-pass vs 404us baseline):
  - Data-parallel over rows of x across 8 NeuronCores (16384/8 = 2048 rows
    per core).  The weight quant-dequant is SHARDED: each core quantizes
    only its 256-row slice of w (host passes it as "wsh"), transposes the
    dequantized bf16 result k-major, and an AllGather (DRAM bounce buffers,
    CC rings - measured ~free when overlapped) assembles the full 2048x2048
    k-major weight on every core.  This cuts the dominant cost - the
    elementwise quant chain on DVE/ACT - from 16 to 9 row-pair chains/core.
  - TRN fp8e4 has max +-240 (not e4m3fn's 448).  Quantizing v/4 on the TRN
    grid and dequantizing with 4*s reproduces e4m3fn rounding exactly for all
    |v| >= 2^-4; s4 = amax*(4/448) wobbles <=1ulp vs jax's division - the
    bf16 operand rounding dominates the (small) overall error.
  - HW-measured: the PE sustains ~53-150ns per [128x128]@[128x512] bf16
    matmul when matmuls share one lhsT load (kb-outer, j-inner) => the whole
    2048^3 per-core matmul is only ~55-150us of PE time.  The kernel is
    bound by the quant chain (DVE tensor_reduce is 1x-mode) + DMA instead:
      * DVE: all abs-max reduces + all quantizes (+ half the x dequants)
      * ACT: the other dequants (16 per-block scaled copies each) + evicts
      * DMA: SWDGE(gpsimd)=input loads, sync=transposes+out stores,
        scalar=gathered-wT loads
  - The mm phase runs in two chunk-pair waves ([0,1] then [2,3]) so half of
    wT and the early x tiles free mid-pass - consecutive passes overlap
    (timing uses straight-line unrolled passes; collectives crash NRT
    inside For_i hardware loops).
  - Output is evicted to bf16 (halves store traffic; ~0.2% extra rounding,
    rel-err stays ~2.9e-3) and upcast to f32 on the host.
"""

import sys

for _p in ("/opt/trn_rl_repo",):
    if _p not in sys.path:
        sys.path.insert(0, _p)

from contextlib import ExitStack

import numpy as np

import concourse.bass as bass  # noqa: F401  (registers engines)
import concourse.tile as tile
from concourse import bacc, mybir
from concourse.bass_utils import run_bass_kernel_spmd

P = 128
N_CORES = 8
B, T, D, OUT = 4, 4096, 2048, 2048
M_FULL = B * T                # 16384
M_CORE = M_FULL // N_CORES    # 2048 rows of x per core

SKIP_MM = False

# ablation knobs: progressively disable pipeline stages (timing probes)
DO_QUANT = True
DO_DEQ = True
DO_TP = True

# engine knobs (tunable): which engine runs quant / deq per stream.
# "mix" alternates vector/scalar by pair index.
X_QUANT = "vector"
X_DEQ = "mix53"
W_QUANT = "vector"
W_DEQ = "scalar"
STORE_ENGINE = "sync"

# W_SHARD: each core quantizes only its 256-row slice of w (input "wsh"),
# then AllGathers the dequantized k-major bf16 weight via DRAM bounce
# buffers.  Cuts the replicated w quant chain 8x.
W_SHARD = True
GATHER_FAKE = False   # timing probe: replace AllGather with local copies
WAVES = True          # W_SHARD: split mms into chunk-pair waves [0,1]/[2,3]
DQP_BUFS = 5          # dequant-tile pool depth
OUTP_BUFS = 6         # output-chunk pool depth


def _resolve(plan, idx):
    if plan == "mix":
        return "vector" if idx % 2 == 0 else "scalar"
    if plan == "mix53":          # 3 of 8 pairs on DVE, 5 on ACT
        return "vector" if idx % 8 in (0, 3, 6) else "scalar"
    return plan


def build(nc, M, K, N, FREE=512, reps=1):
    f32 = mybir.dt.float32
    bf16 = mybir.dt.bfloat16
    fp8 = mybir.dt.float8e4

    KB = K // P     # 16 k-blocks (quant blocks == matmul k-tiles)
    NJ = N // FREE  # 4 psum column chunks
    NX = M // P // 2   # 8 x row-pairs
    NW = N // P // 2   # 8 w row-pairs

    x_d = nc.dram_tensor("x", [M, K], f32, kind="ExternalInput").ap()
    if W_SHARD:
        wsh_d = nc.dram_tensor("wsh", [2 * P, K], f32, kind="ExternalInput").ap()
    else:
        w_d = nc.dram_tensor("w", [N, K], f32, kind="ExternalInput").ap()
    o_d = nc.dram_tensor("out", [M, N], bf16, kind="ExternalOutput").ap()

    with tile.TileContext(nc) as tc, ExitStack() as ctx:
        raw = ctx.enter_context(tc.tile_pool(name="raw", bufs=2))
        qp = ctx.enter_context(tc.tile_pool(name="qp", bufs=3))
        dqp = ctx.enter_context(tc.tile_pool(name="dqp", bufs=DQP_BUFS))
        scl = ctx.enter_context(tc.tile_pool(name="scl", bufs=4))
        wTp = ctx.enter_context(tc.tile_pool(name="wTp", bufs=1))
        xTp = ctx.enter_context(tc.tile_pool(name="xTp", bufs=14))
        outp = ctx.enter_context(tc.tile_pool(name="outp", bufs=OUTP_BUFS))
        psum = ctx.enter_context(tc.tile_pool(name="psum", bufs=2, space="PSUM"))
        if W_SHARD:
            wlp = ctx.enter_context(tc.tile_pool(name="wlp", bufs=1))
            dram = ctx.enter_context(tc.tile_pool(name="dram", bufs=2, space="DRAM"))

        x_d3 = x_d.rearrange("(t p) k -> t p k", p=P)
        if W_SHARD:
            wsh_d3 = wsh_d.rearrange("(t p) k -> t p k", p=P)
        else:
            w_d3 = w_d.rearrange("(t p) k -> t p k", p=P)
        o_d3 = o_d.rearrange("(t p) n -> t p n", p=P)

        def quant_dequant_pair(src_ap, quant_engine, deq_engine):
            """DMA a [128, 2, K] f32 pair of row-tiles (one 2 MB SWDGE
            transfer), blockwise quant-dequant both -> two [P, KB, P] bf16
            tiles."""
            rawt = raw.tile([P, 2, K], f32, tag="raw")
            nc.gpsimd.dma_start(rawt[:], src_ap.rearrange("t p k -> p t k"))
            r4 = rawt[:].rearrange("p t (b q) -> p t b q", q=P)

            amax = scl.tile([P, 2, KB], f32, tag="amax")
            nc.vector.tensor_reduce(
                amax[:], r4, axis=mybir.AxisListType.X,
                op=mybir.AluOpType.max, apply_absolute_value=True,
            )
            s4 = scl.tile([P, 2, KB], f32, tag="s4")
            nc.vector.tensor_scalar(
                s4[:], amax[:], 4.0 / 448.0, None, op0=mybir.AluOpType.mult,
            )
            rinv4 = scl.tile([P, 2, KB], f32, tag="rinv4")
            nc.vector.reciprocal(rinv4[:], s4[:])

            if not DO_QUANT:
                return None
            qt = qp.tile([P, 2, KB, P], fp8, tag="qt")
            if quant_engine == "scalar":
                for t in range(2):
                    for b_ in range(KB):
                        nc.scalar.mul(
                            qt[:, t, b_], r4[:, t, b_], rinv4[:, t, b_ : b_ + 1]
                        )
            else:
                nc.vector.tensor_tensor(
                    qt[:], r4,
                    rinv4[:, :, :, None].broadcast_to((P, 2, KB, P)),
                    op=mybir.AluOpType.mult,
                )
            if not DO_DEQ:
                return None
            outs = []
            for t in range(2):
                dqt = dqp.tile([P, KB, P], bf16, tag="dqt")
                if deq_engine == "scalar":
                    for b_ in range(KB):
                        nc.scalar.mul(
                            dqt[:, b_], qt[:, t, b_], s4[:, t, b_ : b_ + 1]
                        )
                else:
                    nc.vector.tensor_tensor(
                        dqt[:], qt[:, t],
                        s4[:, t, :, None].broadcast_to((P, KB, P)),
                        op=mybir.AluOpType.mult,
                    )
                outs.append(dqt)
            return outs

        def one_pass(rep):
            wT = [
                wTp.tile([P, FREE // P, KB, P], bf16, tag=f"wT{j}",
                         name=f"wT{j}_{rep}")
                for j in range(NJ)
            ]

            def w_pair(wp):
                dqts = quant_dequant_pair(
                    w_d3[2 * wp : 2 * wp + 2],
                    _resolve(W_QUANT, wp), _resolve(W_DEQ, wp))
                if not (DO_DEQ and DO_TP):
                    return
                for t in range(2):
                    wt = 2 * wp + t
                    j, jj = wt // (FREE // P), wt % (FREE // P)
                    nc.sync.dma_start_transpose(wT[j][:, jj], dqts[t][:])

            def w_local():
                """W_SHARD path: quant+transpose this core's 2 w row-tiles,
                AllGather the k-major bf16 result via DRAM, load full wT."""
                dqts = quant_dequant_pair(
                    wsh_d3[0:2], _resolve(W_QUANT, 0), _resolve(W_DEQ, 0))
                if not (DO_DEQ and DO_TP):
                    return
                wTl = wlp.tile([P, 2, KB, P], bf16, tag="wTl",
                               name=f"wTl_{rep}")
                for t in range(2):
                    nc.sync.dma_start_transpose(wTl[:, t], dqts[t][:])
                sh_elems = 2 * KB * P
                gin = dram.tile([P, sh_elems], bf16, tag="gin",
                                name=f"gin_{rep}")
                gout = dram.tile([N_CORES * P, sh_elems], bf16, tag="gout",
                                 name=f"gout_{rep}")
                nc.sync.dma_start(
                    gin[:], wTl[:].rearrange("p t b q -> p (t b q)"))
                if GATHER_FAKE:
                    for c in range(N_CORES):
                        nc.gpsimd.dma_start(
                            gout[c * P : (c + 1) * P, :], gin[:])
                else:
                    nc.gpsimd.collective_compute(
                        "AllGather", mybir.AluOpType.bypass,
                        replica_groups=[list(range(N_CORES))],
                        ins=[gin[:].opt()], outs=[gout[:].opt()],
                    )
                for c in range(N_CORES):
                    j = (2 * c) // (FREE // P)
                    sl = (2 * c) % (FREE // P)
                    nc.scalar.dma_start(
                        wT[j][:, sl : sl + 2],
                        gout[c * P : (c + 1) * P, :].rearrange(
                            "p (t b q) -> p t b q", t=2, q=P),
                    )

            def x_prep(mp):
                dqts = quant_dequant_pair(
                    x_d3[2 * mp : 2 * mp + 2],
                    _resolve(X_QUANT, mp), _resolve(X_DEQ, mp))
                if not (DO_DEQ and DO_TP):
                    return None
                pair = []
                for t in range(2):
                    xT = xTp.tile([P, KB, P], bf16, tag="xT",
                                  name=f"xT_{rep}_{mp}_{t}")
                    nc.sync.dma_start_transpose(xT[:], dqts[t][:])
                    pair.append(xT)
                return pair

            def x_mm(mp, xTs, js):
                """Matmul groups for row-pair mp over chunk set js
                (kb-outer, j-inner: js matmuls share each lhsT load)."""
                if SKIP_MM or xTs is None:
                    return
                for t in range(2):
                    mt = 2 * mp + t
                    pst = {
                        j: psum.tile([P, FREE], f32, tag=f"ps{j}",
                                     name=f"ps{j}_{rep}_{mt}")
                        for j in js
                    }
                    for kb in range(KB):
                        for j in js:
                            nc.tensor.matmul(
                                pst[j][:], lhsT=xTs[t][:, kb, :],
                                rhs=wT[j][:, :, kb, :],
                                start=(kb == 0), stop=(kb == KB - 1),
                            )
                    outc = outp.tile([P, len(js), FREE], bf16, tag="outt",
                                     name=f"oc_{rep}_{mt}_{js[0]}")
                    for i, j in enumerate(js):
                        nc.scalar.copy(outc[:, i], pst[j][:])
                    store_eng = nc.sync if STORE_ENGINE == "sync" else nc.scalar
                    store_eng.dma_start(
                        o_d3[mt, :, js[0] * FREE : (js[-1] + 1) * FREE],
                        outc[:].rearrange("p c f -> p (c f)"),
                    )

            if W_SHARD:
                # w chain is 1 pair + AllGather; stream x pairs.  WAVES
                # splits the mm phase so wT[0:2] (and the x tiles) free
                # mid-pass, letting consecutive unrolled passes overlap.
                w_local()
                if WAVES:
                    pre = {}
                    for mp in range(NX):
                        pre[mp] = x_prep(mp)
                        x_mm(mp, pre[mp], [0, 1])
                    for mp in range(NX):
                        x_mm(mp, pre.pop(mp), [2, 3])
                else:
                    for mp in range(NX):
                        x_mm(mp, x_prep(mp), [0, 1, 2, 3])
                return
            # Emission: interleave x preps, w chunks, and matmul waves so
            # the PE starts after 2 w-pairs and DVE/ACT never idle.
            # wT chunk j is complete after w pairs 2j, 2j+1.
            pre = {}
            pre[0] = x_prep(0)
            w_pair(0); w_pair(1)                      # chunk 0
            x_mm(0, pre[0], [0])
            w_pair(2); w_pair(3)                      # chunk 1
            pre[1] = x_prep(1)
            x_mm(1, pre[1], [0])
            x_mm(0, pre[0], [1]); x_mm(1, pre[1], [1])
            w_pair(4); w_pair(5)                      # chunk 2
            pre[2] = x_prep(2)
            x_mm(2, pre[2], [0, 1])
            x_mm(0, pre[0], [2]); x_mm(1, pre[1], [2]); x_mm(2, pre[2], [2])
            w_pair(6); w_pair(7)                      # chunk 3
            pre[3] = x_prep(3)
            x_mm(3, pre[3], [0, 1]); x_mm(3, pre[3], [2])
            x_mm(0, pre.pop(0), [3]); x_mm(1, pre.pop(1), [3])
            x_mm(2, pre.pop(2), [3]); x_mm(3, pre.pop(3), [3])
            for mp in range(4, NX):
                x_mm(mp, x_prep(mp), [0, 1, 2, 3])

        if reps == 1:
            one_pass(0)
        elif W_SHARD:
            # Collectives crash NRT inside For_i hardware loops; emit the
            # passes straight-line (also overlaps pass tails/heads).
            for r in range(reps):
                one_pass(r)
        else:
            with tc.For_i(0, reps, 1):
                one_pass(0)

    return nc


_NCS = {}


def _get_nc(reps=1):
    if reps not in _NCS:
        nc = bacc.Bacc(
            "TRN2", target_bir_lowering=False, debug=False,
            enable_asserts=False, num_devices=N_CORES,
        )
        build(nc, M_CORE, D, OUT, reps=reps)
        nc.compile()
        _NCS[reps] = nc
    return _NCS[reps]


def _in_maps(x, weight):
    x2 = np.ascontiguousarray(
        np.asarray(x, dtype=np.float32).reshape(M_FULL, D)
    )
    w = np.ascontiguousarray(np.asarray(weight, dtype=np.float32))
    wpc = OUT // N_CORES  # w rows quantized per core under W_SHARD
    if W_SHARD:
        return [
            {"x": x2[c * M_CORE : (c + 1) * M_CORE],
             "wsh": np.ascontiguousarray(w[c * wpc : (c + 1) * wpc])}
            for c in range(N_CORES)
        ]
    return [
        {"x": x2[c * M_CORE : (c + 1) * M_CORE], "w": w}
        for c in range(N_CORES)
    ]


def kernel(x, weight):
    nc = _get_nc()
    res = run_bass_kernel_spmd(nc, _in_maps(x, weight), core_ids=list(range(N_CORES)))
    out = np.concatenate(
        [np.asarray(res.results[c]["out"]).astype(np.float32)
         for c in range(N_CORES)],
        axis=0,
    )
    return out.reshape(B, T, OUT)


class _Runner:
    """Reusable jitted single-NEFF-execution runner (device-resident inputs)."""

    def __init__(self, nc):
        import jax
        from jax.experimental.shard_map import shard_map
        from jax.sharding import Mesh, NamedSharding, PartitionSpec

        from concourse import bass2jax

        bass2jax.install_neuronx_cc_hook()
        self.jax = jax
        self.nc = nc

        in_names, out_names, out_avals = [], [], []
        self.out_np_dtype = None
        for alloc in nc.m.functions[0].allocations:
            if not isinstance(alloc, mybir.MemoryLocationSet):
                continue
            name = alloc.memorylocations[0].name
            if alloc.kind == "ExternalInput":
                in_names.append(name)
            elif alloc.kind == "ExternalOutput":
                out_names.append(name)
                self.out_np_dtype = mybir.dt.np(alloc.dtype)
                out_avals.append(
                    jax.core.ShapedArray(
                        tuple(alloc.tensor_shape), mybir.dt.np(alloc.dtype)
                    )
                )
        partition_name = (
            nc.partition_id_tensor.name if nc.partition_id_tensor else None
        )
        in_names = [n for n in in_names if n != partition_name]
        assert sorted(in_names) in (["w", "x"], ["wsh", "x"]), in_names
        assert out_names == ["out"]
        self.in_names = in_names
        all_names = in_names + out_names
        if partition_name is not None:
            all_names.append(partition_name)
        all_names = tuple(all_names)

        def _exec(ina, inb, za):
            operands = [ina, inb, za]
            if partition_name is not None:
                operands.append(bass2jax.partition_id_tensor())
            outs = bass2jax._bass_exec_p.bind(
                *operands,
                out_avals=tuple(out_avals),
                in_names=all_names,
                out_names=tuple(out_names),
                lowering_input_output_aliases=(),
                sim_require_finite=True,
                sim_require_nnan=True,
                nc=nc,
            )
            return (outs[0],)

        devices = jax.devices()[:N_CORES]
        self.mesh = Mesh(np.asarray(devices), ("core",))
        self.sharding = NamedSharding(self.mesh, PartitionSpec("core"))
        self.fn = jax.jit(
            shard_map(
                _exec, mesh=self.mesh,
                in_specs=(PartitionSpec("core"),) * 3,
                out_specs=(PartitionSpec("core"),),
                check_rep=False,
            ),
            donate_argnums=(2,),
            keep_unused=True,
        )
        odt = self.out_np_dtype
        self.zfn = jax.jit(
            lambda: jax.numpy.zeros((M_FULL, OUT), odt),
            out_shardings=self.sharding,
        )

    def put_inputs(self, x, weight):
        maps = _in_maps(x, weight)
        self.dins = [
            self.jax.device_put(
                np.concatenate([m[n] for m in maps], axis=0), self.sharding)
            for n in self.in_names
        ]

    def run(self):
        out = self.fn(*self.dins, self.zfn())[0]
        self.jax.block_until_ready(out)
        return out

    def time_min(self, n=5):
        import time

        best = float("inf")
        for _ in range(n):
            z = self.zfn()
            self.jax.block_until_ready(z)
            t0 = time.perf_counter()
            out = self.fn(*self.dins, z)[0]
            self.jax.block_until_ready(out)
            best = min(best, time.perf_counter() - t0)
        return best


def kernel_timed(x, weight, reps=None, n=6):
    """Returns (out, est_exec_ns) via the (T_R - T_r)/(R-r) delta method."""
    lo = 3 if W_SHARD else 1   # W_SHARD passes are unrolled; cap compile size
    reps = reps or (13 if W_SHARD else 64)
    r1 = _Runner(_get_nc(lo))
    r1.put_inputs(x, weight)
    out = np.asarray(r1.run()).astype(np.float32)
    t1 = r1.time_min(n)

    rR = _Runner(_get_nc(reps))
    rR.put_inputs(x, weight)
    outR = np.asarray(rR.run()).astype(np.float32)
    tR = rR.time_min(n)
    assert np.array_equal(out, outR), "reps variant disagrees"

    est_ns = (tR - t1) / (reps - lo) * 1e9
    print(f"[timing] T{lo}={t1*1e3:.3f} ms  T{reps}={tR*1e3:.3f} ms  "
          f"delta/iter={est_ns/1e3:.1f} us")
    return out.reshape(B, T, OUT), int(est_ns)


# revision 46
# speedup vs baseline: 1.2134x; 1.2134x over previous
"""FP8Linear (blockwise fp8 quant-dequant + matmul) Trainium2 Bass kernel.

Reference semantics (per 128-contiguous-element block, flattened):
    s = max|block| / 448 ; q = fp8_e4m3fn(block / s) ; deq = q * s
    out = x_deq @ w_deq.T

Strategy (v5, ~160us/core-pass vs 404us baseline):
  - Data-parallel over rows of x across 8 NeuronCores (16384/8 = 2048 rows
    per core).  The weight quant-dequant is SHARDED: each core quantizes
    only its 256-row slice of w (host passes it as "wsh"), transposes the
    dequantized bf16 result k-major, and an AllGather (DRAM bounce buffers,
    CC rings - measured ~free when overlapped) assembles the full 2048x2048
    k-major weight on every core.  This cuts the dominant cost - the
    elementwise quant chain on DVE/ACT - from 16 to 9 row-pair chains/core.
  - TRN fp8e4 has max +-240 (not e4m3fn's 448).  Quantizing v/4 on the TRN
    grid and dequantizing with 4*s reproduces e4m3fn rounding exactly for all
    |v| >= 2^-4; s4 = amax*(4/448) wobbles <=1ulp vs jax's division - the
    bf16 operand rounding dominates the (small) overall error.
  - HW-measured: the PE sustains ~53-150ns per [128x128]@[128x512] bf16
    matmul when matmuls share one lhsT load (kb-outer, j-inner) => the whole
    2048^3 per-core matmul is only ~55-150us of PE time.  The kernel is
    bound by the quant chain (DVE tensor_reduce is 1x-mode) + DMA instead:
      * DVE: all abs-max reduces + all quantizes (+ half the x dequants)
      * ACT: the other dequants (16 per-block scaled copies each) + evicts
      * DMA: SWDGE(gpsimd)=input loads, sync=transposes+out stores,
        scalar=gathered-wT loads
  - The mm phase runs in two chunk-pair waves ([0,1] then [2,3]) so half of
    wT and the early x tiles free mid-pass - consecutive passes overlap
    (timing uses straight-line unrolled passes; collectives crash NRT
    inside For_i hardware loops).
  - Output is evicted to bf16 (halves store traffic; ~0.2% extra rounding,
    rel-err stays ~2.9e-3) and upcast to f32 on the host.
"""

import sys

for _p in ("/opt/trn_rl_repo",):
    if _p not in sys.path:
        sys.path.insert(0, _p)

from contextlib import ExitStack

import numpy as np

import concourse.bass as bass  # noqa: F401  (registers engines)
import concourse.tile as tile
from concourse import bacc, mybir
from concourse.bass_utils import run_bass_kernel_spmd

P = 128
N_CORES = 8
B, T, D, OUT = 4, 4096, 2048, 2048
M_FULL = B * T                # 16384
M_CORE = M_FULL // N_CORES    # 2048 rows of x per core

SKIP_MM = False

# ablation knobs: progressively disable pipeline stages (timing probes)
DO_QUANT = True
DO_DEQ = True
DO_TP = True

# engine knobs (tunable): which engine runs quant / deq per stream.
# "mix" alternates vector/scalar by pair index.
X_QUANT = "vector"
X_DEQ = "mix53"
W_QUANT = "vector"
W_DEQ = "scalar"
STORE_ENGINE = "sync"

# W_SHARD: each core quantizes only its 256-row slice of w (input "wsh"),
# then AllGathers the dequantized k-major bf16 weight via DRAM bounce
# buffers.  Cuts the replicated w quant chain 8x.
W_SHARD = True
GATHER_FAKE = False   # timing probe: replace AllGather with local copies
WAVES = True          # W_SHARD: split mms into chunk-pair waves [0,1]/[2,3]
DQP_BUFS = 5          # dequant-tile pool depth
OUTP_BUFS = 6         # output-chunk pool depth
SCL_BUFS = 8          # scale-tile pool depth (s4 held until deq completes;
                      # 4 slots WAR-gate DVE's FIFO behind the 5 late ACT deqs)
QP_BUFS = 3           # quantized-tile pool depth (qt held until deq)
XT_BUFS = 14          # transposed-x pool depth


def _resolve(plan, idx):
    if plan == "mix":
        return "vector" if idx % 2 == 0 else "scalar"
    if plan == "mix53":          # 3 of 8 pairs on DVE, 5 on ACT
        return "vector" if idx % 8 in (0, 3, 6) else "scalar"
    return plan


def build(nc, M, K, N, FREE=512, reps=1):
    f32 = mybir.dt.float32
    bf16 = mybir.dt.bfloat16
    fp8 = mybir.dt.float8e4

    KB = K // P     # 16 k-blocks (quant blocks == matmul k-tiles)
    NJ = N // FREE  # 4 psum column chunks
    NX = M // P // 2   # 8 x row-pairs
    NW = N // P // 2   # 8 w row-pairs

    x_d = nc.dram_tensor("x", [M, K], f32, kind="ExternalInput").ap()
    if W_SHARD:
        wsh_d = nc.dram_tensor("wsh", [2 * P, K], f32, kind="ExternalInput").ap()
    else:
        w_d = nc.dram_tensor("w", [N, K], f32, kind="ExternalInput").ap()
    o_d = nc.dram_tensor("out", [M, N], bf16, kind="ExternalOutput").ap()

    with tile.TileContext(nc) as tc, ExitStack() as ctx:
        raw = ctx.enter_context(tc.tile_pool(name="raw", bufs=2))
        qp = ctx.enter_context(tc.tile_pool(name="qp", bufs=QP_BUFS))
        dqp = ctx.enter_context(tc.tile_pool(name="dqp", bufs=DQP_BUFS))
        scl = ctx.enter_context(tc.tile_pool(name="scl", bufs=SCL_BUFS))
        wTp = ctx.enter_context(tc.tile_pool(name="wTp", bufs=1))
        xTp = ctx.enter_context(tc.tile_pool(name="xTp", bufs=XT_BUFS))
        outp = ctx.enter_context(tc.tile_pool(name="outp", bufs=OUTP_BUFS))
        psum = ctx.enter_context(tc.tile_pool(name="psum", bufs=2, space="PSUM"))
        if W_SHARD:
            wlp = ctx.enter_context(tc.tile_pool(name="wlp", bufs=1))
            dram = ctx.enter_context(tc.tile_pool(name="dram", bufs=2, space="DRAM"))

        x_d3 = x_d.rearrange("(t p) k -> t p k", p=P)
        if W_SHARD:
            wsh_d3 = wsh_d.rearrange("(t p) k -> t p k", p=P)
        else:
            w_d3 = w_d.rearrange("(t p) k -> t p k", p=P)
        o_d3 = o_d.rearrange("(t p) n -> t p n", p=P)

        def quant_dequant_pair(src_ap, quant_engine, deq_engine):
            """DMA a [128, 2, K] f32 pair of row-tiles (one 2 MB SWDGE
            transfer), blockwise quant-dequant both -> two [P, KB, P] bf16
            tiles."""
            rawt = raw.tile([P, 2, K], f32, tag="raw")
            nc.gpsimd.dma_start(rawt[:], src_ap.rearrange("t p k -> p t k"))
            r4 = rawt[:].rearrange("p t (b q) -> p t b q", q=P)

            amax = scl.tile([P, 2, KB], f32, tag="amax")
            nc.vector.tensor_reduce(
                amax[:], r4, axis=mybir.AxisListType.X,
                op=mybir.AluOpType.max, apply_absolute_value=True,
            )
            s4 = scl.tile([P, 2, KB], f32, tag="s4")
            nc.vector.tensor_scalar(
                s4[:], amax[:], 4.0 / 448.0, None, op0=mybir.AluOpType.mult,
            )
            rinv4 = scl.tile([P, 2, KB], f32, tag="rinv4")
            nc.vector.reciprocal(rinv4[:], s4[:])

            if not DO_QUANT:
                return None
            qt = qp.tile([P, 2, KB, P], fp8, tag="qt")
            if quant_engine == "scalar":
                for t in range(2):
                    for b_ in range(KB):
                        nc.scalar.mul(
                            qt[:, t, b_], r4[:, t, b_], rinv4[:, t, b_ : b_ + 1]
                        )
            else:
                nc.vector.tensor_tensor(
                    qt[:], r4,
                    rinv4[:, :, :, None].broadcast_to((P, 2, KB, P)),
                    op=mybir.AluOpType.mult,
                )
            if not DO_DEQ:
                return None
            outs = []
            for t in range(2):
                dqt = dqp.tile([P, KB, P], bf16, tag="dqt")
                if deq_engine == "scalar":
                    for b_ in range(KB):
                        nc.scalar.mul(
                            dqt[:, b_], qt[:, t, b_], s4[:, t, b_ : b_ + 1]
                        )
                else:
                    nc.vector.tensor_tensor(
                        dqt[:], qt[:, t],
                        s4[:, t, :, None].broadcast_to((P, KB, P)),
                        op=mybir.AluOpType.mult,
                    )
                outs.append(dqt)
            return outs

        def one_pass(rep):
            wT = [
                wTp.tile([P, FREE // P, KB, P], bf16, tag=f"wT{j}",
                         name=f"wT{j}_{rep}")
                for j in range(NJ)
            ]

            def w_pair(wp):
                dqts = quant_dequant_pair(
                    w_d3[2 * wp : 2 * wp + 2],
                    _resolve(W_QUANT, wp), _resolve(W_DEQ, wp))
                if not (DO_DEQ and DO_TP):
                    return
                for t in range(2):
                    wt = 2 * wp + t
                    j, jj = wt // (FREE // P), wt % (FREE // P)
                    nc.sync.dma_start_transpose(wT[j][:, jj], dqts[t][:])

            def w_local():
                """W_SHARD path: quant+transpose this core's 2 w row-tiles,
                AllGather the k-major bf16 result via DRAM, load full wT."""
                dqts = quant_dequant_pair(
                    wsh_d3[0:2], _resolve(W_QUANT, 0), _resolve(W_DEQ, 0))
                if not (DO_DEQ and DO_TP):
                    return
                wTl = wlp.tile([P, 2, KB, P], bf16, tag="wTl",
                               name=f"wTl_{rep}")
                for t in range(2):
                    nc.sync.dma_start_transpose(wTl[:, t], dqts[t][:])
                sh_elems = 2 * KB * P
                gin = dram.tile([P, sh_elems], bf16, tag="gin",
                                name=f"gin_{rep}")
                gout = dram.tile([N_CORES * P, sh_elems], bf16, tag="gout",
                                 name=f"gout_{rep}")
                nc.sync.dma_start(
                    gin[:], wTl[:].rearrange("p t b q -> p (t b q)"))
                if GATHER_FAKE:
                    for c in range(N_CORES):
                        nc.gpsimd.dma_start(
                            gout[c * P : (c + 1) * P, :], gin[:])
                else:
                    nc.gpsimd.collective_compute(
                        "AllGather", mybir.AluOpType.bypass,
                        replica_groups=[list(range(N_CORES))],
                        ins=[gin[:].opt()], outs=[gout[:].opt()],
                    )
                for c in range(N_CORES):
                    j = (2 * c) // (FREE // P)
                    sl = (2 * c) % (FREE // P)
                    nc.scalar.dma_start(
                        wT[j][:, sl : sl + 2],
                        gout[c * P : (c + 1) * P, :].rearrange(
                            "p (t b q) -> p t b q", t=2, q=P),
                    )

            def x_prep(mp):
                dqts = quant_dequant_pair(
                    x_d3[2 * mp : 2 * mp + 2],
                    _resolve(X_QUANT, mp), _resolve(X_DEQ, mp))
                if not (DO_DEQ and DO_TP):
                    return None
                pair = []
                for t in range(2):
                    xT = xTp.tile([P, KB, P], bf16, tag="xT",
                                  name=f"xT_{rep}_{mp}_{t}")
                    nc.sync.dma_start_transpose(xT[:], dqts[t][:])
                    pair.append(xT)
                return pair

            def x_mm(mp, xTs, js):
                """Matmul groups for row-pair mp over chunk set js
                (kb-outer, j-inner: js matmuls share each lhsT load)."""
                if SKIP_MM or xTs is None:
                    return
                for t in range(2):
                    mt = 2 * mp + t
                    pst = {
                        j: psum.tile([P, FREE], f32, tag=f"ps{j}",
                                     name=f"ps{j}_{rep}_{mt}")
                        for j in js
                    }
                    for kb in range(KB):
                        for j in js:
                            nc.tensor.matmul(
                                pst[j][:], lhsT=xTs[t][:, kb, :],
                                rhs=wT[j][:, :, kb, :],
                                start=(kb == 0), stop=(kb == KB - 1),
                            )
                    outc = outp.tile([P, len(js), FREE], bf16, tag="outt",
                                     name=f"oc_{rep}_{mt}_{js[0]}")
                    for i, j in enumerate(js):
                        nc.scalar.copy(outc[:, i], pst[j][:])
                    store_eng = nc.sync if STORE_ENGINE == "sync" else nc.scalar
                    store_eng.dma_start(
                        o_d3[mt, :, js[0] * FREE : (js[-1] + 1) * FREE],
                        outc[:].rearrange("p c f -> p (c f)"),
                    )

            if W_SHARD:
                # w chain is 1 pair + AllGather; stream x pairs.  WAVES
                # splits the mm phase so wT[0:2] (and the x tiles) free
                # mid-pass, letting consecutive unrolled passes overlap.
                w_local()
                if WAVES:
                    pre = {}
                    for mp in range(NX):
                        pre[mp] = x_prep(mp)
                        x_mm(mp, pre[mp], [0, 1])
                    for mp in range(NX):
                        x_mm(mp, pre.pop(mp), [2, 3])
                else:
                    for mp in range(NX):
                        x_mm(mp, x_prep(mp), [0, 1, 2, 3])
                return
            # Emission: interleave x preps, w chunks, and matmul waves so
            # the PE starts after 2 w-pairs and DVE/ACT never idle.
            # wT chunk j is complete after w pairs 2j, 2j+1.
            pre = {}
            pre[0] = x_prep(0)
            w_pair(0); w_pair(1)                      # chunk 0
            x_mm(0, pre[0], [0])
            w_pair(2); w_pair(3)                      # chunk 1
            pre[1] = x_prep(1)
            x_mm(1, pre[1], [0])
            x_mm(0, pre[0], [1]); x_mm(1, pre[1], [1])
            w_pair(4); w_pair(5)                      # chunk 2
            pre[2] = x_prep(2)
            x_mm(2, pre[2], [0, 1])
            x_mm(0, pre[0], [2]); x_mm(1, pre[1], [2]); x_mm(2, pre[2], [2])
            w_pair(6); w_pair(7)                      # chunk 3
            pre[3] = x_prep(3)
            x_mm(3, pre[3], [0, 1]); x_mm(3, pre[3], [2])
            x_mm(0, pre.pop(0), [3]); x_mm(1, pre.pop(1), [3])
            x_mm(2, pre.pop(2), [3]); x_mm(3, pre.pop(3), [3])
            for mp in range(4, NX):
                x_mm(mp, x_prep(mp), [0, 1, 2, 3])

        if reps == 1:
            one_pass(0)
        elif W_SHARD:
            # Collectives crash NRT inside For_i hardware loops; emit the
            # passes straight-line (also overlaps pass tails/heads).
            for r in range(reps):
                one_pass(r)
        else:
            with tc.For_i(0, reps, 1):
                one_pass(0)

    return nc


_NCS = {}


def _get_nc(reps=1):
    if reps not in _NCS:
        nc = bacc.Bacc(
            "TRN2", target_bir_lowering=False, debug=False,
            enable_asserts=False, num_devices=N_CORES,
        )
        build(nc, M_CORE, D, OUT, reps=reps)
        nc.compile()
        _NCS[reps] = nc
    return _NCS[reps]


def _in_maps(x, weight):
    x2 = np.ascontiguousarray(
        np.asarray(x, dtype=np.float32).reshape(M_FULL, D)
    )
    w = np.ascontiguousarray(np.asarray(weight, dtype=np.float32))
    wpc = OUT // N_CORES  # w rows quantized per core under W_SHARD
    if W_SHARD:
        return [
            {"x": x2[c * M_CORE : (c + 1) * M_CORE],
             "wsh": np.ascontiguousarray(w[c * wpc : (c + 1) * wpc])}
            for c in range(N_CORES)
        ]
    return [
        {"x": x2[c * M_CORE : (c + 1) * M_CORE], "w": w}
        for c in range(N_CORES)
    ]


def kernel(x, weight):
    nc = _get_nc()
    res = run_bass_kernel_spmd(nc, _in_maps(x, weight), core_ids=list(range(N_CORES)))
    out = np.concatenate(
        [np.asarray(res.results[c]["out"]).astype(np.float32)
         for c in range(N_CORES)],
        axis=0,
    )
    return out.reshape(B, T, OUT)


class _Runner:
    """Reusable jitted single-NEFF-execution runner (device-resident inputs)."""

    def __init__(self, nc):
        import jax
        from jax.experimental.shard_map import shard_map
        from jax.sharding import Mesh, NamedSharding, PartitionSpec

        from concourse import bass2jax

        bass2jax.install_neuronx_cc_hook()
        self.jax = jax
        self.nc = nc

        in_names, out_names, out_avals = [], [], []
        self.out_np_dtype = None
        for alloc in nc.m.functions[0].allocations:
            if not isinstance(alloc, mybir.MemoryLocationSet):
                continue
            name = alloc.memorylocations[0].name
            if alloc.kind == "ExternalInput":
                in_names.append(name)
            elif alloc.kind == "ExternalOutput":
                out_names.append(name)
                self.out_np_dtype = mybir.dt.np(alloc.dtype)
                out_avals.append(
                    jax.core.ShapedArray(
                        tuple(alloc.tensor_shape), mybir.dt.np(alloc.dtype)
                    )
                )
        partition_name = (
            nc.partition_id_tensor.name if nc.partition_id_tensor else None
        )
        in_names = [n for n in in_names if n != partition_name]
        assert sorted(in_names) in (["w", "x"], ["wsh", "x"]), in_names
        assert out_names == ["out"]
        self.in_names = in_names
        all_names = in_names + out_names
        if partition_name is not None:
            all_names.append(partition_name)
        all_names = tuple(all_names)

        def _exec(ina, inb, za):
            operands = [ina, inb, za]
            if partition_name is not None:
                operands.append(bass2jax.partition_id_tensor())
            outs = bass2jax._bass_exec_p.bind(
                *operands,
                out_avals=tuple(out_avals),
                in_names=all_names,
                out_names=tuple(out_names),
                lowering_input_output_aliases=(),
                sim_require_finite=True,
                sim_require_nnan=True,
                nc=nc,
            )
            return (outs[0],)

        devices = jax.devices()[:N_CORES]
        self.mesh = Mesh(np.asarray(devices), ("core",))
        self.sharding = NamedSharding(self.mesh, PartitionSpec("core"))
        self.fn = jax.jit(
            shard_map(
                _exec, mesh=self.mesh,
                in_specs=(PartitionSpec("core"),) * 3,
                out_specs=(PartitionSpec("core"),),
                check_rep=False,
            ),
            donate_argnums=(2,),
            keep_unused=True,
        )
        odt = self.out_np_dtype
        self.zfn = jax.jit(
            lambda: jax.numpy.zeros((M_FULL, OUT), odt),
            out_shardings=self.sharding,
        )

    def put_inputs(self, x, weight):
        maps = _in_maps(x, weight)
        self.dins = [
            self.jax.device_put(
                np.concatenate([m[n] for m in maps], axis=0), self.sharding)
            for n in self.in_names
        ]

    def run(self):
        out = self.fn(*self.dins, self.zfn())[0]
        self.jax.block_until_ready(out)
        return out

    def time_min(self, n=5):
        import time

        best = float("inf")
        for _ in range(n):
            z = self.zfn()
            self.jax.block_until_ready(z)
            t0 = time.perf_counter()
            out = self.fn(*self.dins, z)[0]
            self.jax.block_until_ready(out)
            best = min(best, time.perf_counter() - t0)
        return best


def kernel_timed(x, weight, reps=None, n=6):
    """Returns (out, est_exec_ns) via the (T_R - T_r)/(R-r) delta method."""
    lo = 3 if W_SHARD else 1   # W_SHARD passes are unrolled; cap compile size
    reps = reps or (13 if W_SHARD else 64)
    r1 = _Runner(_get_nc(lo))
    r1.put_inputs(x, weight)
    out = np.asarray(r1.run()).astype(np.float32)
    t1 = r1.time_min(n)

    rR = _Runner(_get_nc(reps))
    rR.put_inputs(x, weight)
    outR = np.asarray(rR.run()).astype(np.float32)
    tR = rR.time_min(n)
    assert np.array_equal(out, outR), "reps variant disagrees"

    est_ns = (tR - t1) / (reps - lo) * 1e9
    print(f"[timing] T{lo}={t1*1e3:.3f} ms  T{reps}={tR*1e3:.3f} ms  "
          f"delta/iter={est_ns/1e3:.1f} us")
    return out.reshape(B, T, OUT), int(est_ns)
